# revision 44
# baseline (speedup 1.0000x reference)
"""Causal GQA varlen-prefill attention on 8 TRN2 NeuronCores.

Problem: B=4 sequences of S=2048, 16 Q heads, 4 KV heads (GQA group 4),
head_dim 128, fp32. Sharded across 8 cores by (batch, kv-head) unit:
16 units, 2 per core - embarrassingly parallel, no collectives.

v2 redesign (from perfetto evidence on the 255us v1):
  - ACT runs ONLY exp, in 13 wide ops per (u,g) instead of 28+32copies+
    64 DMA issues: diagonal-superblock score chunks are bin-packed into
    shared [128,<=1536] PSUM tiles with zero dead columns, so every
    ACTIVATE amortizes its ~300ns fixed cost over 1280-1536 columns.
  - PE runs ONLY the score/PV matmuls (the l-reduction ones-matmul and
    the 1/l broadcast matmul are gone): l is computed by GpSimd
    partition_all_reduce (idle engine), 1/l by the custom-DVE
    reciprocal_approx_fast, and the normalize multiply reads the PSUM
    O^T accumulator directly.
  - Causal tri-masks moved DVE -> GpSimd (in-place on the pt tile).
  - PSUM: 2x[128,1536] score tiles (6 banks) + 2x[128,512] O^T
    accumulators (2 banks) = exactly 8 banks.
  - Tile stream is software-pipelined 2 tiles deep across block
    boundaries (diagonal-chunk consumers one slot later so GpSimd masks
    never gate the PE), and the deferred 1/l normalize of block n is
    emitted at block n+1's consumer entry.

Measured: ~195-199us on hardware (baseline v1: ~255us), rel err 2.9e-3
vs the f32 reference.  Engine busy at nominal clock: PE ~146us union +
~36us dependency gaps (wall-setting), DVE ~175us, ACT ~146us (pure exp
stream), GpSimd ~65us; ~17us is fixed NEFF warmup/tail.  Note: device
clock throttling makes single runs vary up to ~20% - compare traces by
per-op durations, not wall time.
"""

import sys

if "/opt/trn_rl_repo" not in sys.path:
    sys.path.insert(0, "/opt/trn_rl_repo")

import numpy as np
import ml_dtypes

import concourse.bass as bass
import concourse.mybir as mybir
from concourse.bass_utils import run_bass_kernel_spmd
from concourse.tile import TileContext, ScopedClock

B, S, H, HKV, D = 4, 2048, 16, 4, 128
G = H // HKV
NCORES = 8
UNITS = 2            # (b, kv) units per core
SQ = 512             # q-chunk (matmul moving dim)
NQT = S // SQ        # 4 q-chunks per (unit, head)
NKC = S // 128       # 16 k-chunks of 128
SCALE = 1.0 / float(np.sqrt(D))

F32 = mybir.dt.float32
BF16 = mybir.dt.bfloat16
NP_BF16 = np.dtype(ml_dtypes.bfloat16)


def _patched_drain_and_barrier(self, tick_clock, wait_clock):
    # walrus CoreV3 rejects >1 sync-wait on one InstDrain ("Too many sync
    # wait commands"); spread the kernel-tail waits over single-wait nops.
    # Also: skip the per-semaphore clearing ritual + second barrier (the
    # NEFF executes once per load; ~9us of tail EVENT_SEMAPHOREs saved).
    drain_inst = self.nc.sync.drain()
    wait_clock.add_sem_waits(
        drain_inst.ins, ScopedClock({None: tick_clock.global_clock})
    )
    si = drain_inst.ins.sync_info
    waits = list(si.on_wait or [])
    if len(waits) > 1:
        si.on_wait = []
        for w in waits:
            nop = self.nc.sync.nop(nofuse=True)
            nsi = nop.ins.sync_info
            if nsi is None:
                nop.ins.sync_info = mybir.SyncInfo(on_wait=[w], on_update=[])
            else:
                nsi.on_wait = [w]
        self.nc.sync.drain()
    self.nc.all_engine_barrier()
    assert self.sems is not None
    popped = self.nc._tile_sem_poison_stack.pop()
    assert popped is self._sem_poison


TileContext._drain_and_barrier = _patched_drain_and_barrier

_WAIT_LIMIT = 1
_nop_counter = [0]


def _split_multiwait_instructions(nc):
    # This walrus build allows only one sync-wait command per instruction
    # (CoreV3 setupSyncWait: "Too many sync wait commands").  Hoist extra
    # waits onto same-engine nops placed immediately before the instruction.
    for fn in nc.m.functions:
        for bb in fn.blocks:
            new_list = []
            changed = False
            for inst in bb.instructions:
                si = inst.sync_info
                waits = list(si.on_wait) if si is not None and si.on_wait else []
                if len(waits) > _WAIT_LIMIT:
                    keep = waits[-_WAIT_LIMIT:]
                    for w in waits[:-_WAIT_LIMIT]:
                        _nop_counter[0] += 1
                        nop = mybir.InstNoOp(
                            name=f"I-waitnop-{_nop_counter[0]}",
                            engine=inst.engine,
                            ins=[],
                            outs=[],
                            sync_info=mybir.SyncInfo(on_wait=[w], on_update=[]),
                        )
                        nc.register_instruction(nop, overwrite=True)
                        new_list.append(nop)
                    si.on_wait = keep
                    changed = True
                new_list.append(inst)
            if changed:
                bb.instructions = new_list


# Score-chunk packing: per qt, list of PSUM tiles; each tile is a list of
# (kc, sq0, pcol): chunk kc's live q-columns [sq0:512] land at packed
# columns [pcol : pcol + 512-sq0].  Every chunk segment sits inside one
# 512-f32 PSUM bank, the packing is gap-free from column 0, and diagonal
# chunks (kc >= 4qt) get a tri-mask on their leading 128 live columns.
TILES = {
    0: [[(0, 0, 0), (3, 384, 512), (1, 128, 640), (2, 256, 1024)]],
    1: [[(0, 0, 0), (1, 0, 512), (2, 0, 1024)],
        [(3, 0, 0), (4, 0, 512), (5, 128, 1024)],
        [(6, 256, 0), (7, 384, 256)]],
    2: [[(0, 0, 0), (1, 0, 512), (2, 0, 1024)],
        [(3, 0, 0), (4, 0, 512), (5, 0, 1024)],
        [(6, 0, 0), (7, 0, 512), (8, 0, 1024)],
        [(9, 128, 0), (11, 384, 384), (10, 256, 512)]],
    3: [[(0, 0, 0), (1, 0, 512), (2, 0, 1024)],
        [(3, 0, 0), (4, 0, 512), (5, 0, 1024)],
        [(6, 0, 0), (7, 0, 512), (8, 0, 1024)],
        [(9, 0, 0), (10, 0, 512), (11, 0, 1024)],
        [(12, 0, 0), (13, 128, 512), (15, 384, 896), (14, 256, 1024)]],
}


def build_nc() -> bass.Bass:
    nc = bass.Bass()
    qT_ext = nc.declare_dram_parameter("qT", [UNITS, G, D, S], BF16, isOutput=False)
    kT_ext = nc.declare_dram_parameter("kT", [UNITS, D, S], BF16, isOutput=False)
    v_ext = nc.declare_dram_parameter("v", [UNITS, 128, NKC * 128], BF16,
                                      isOutput=False)
    tri_ext = nc.declare_dram_parameter("tri", [128, 128], BF16, isOutput=False)
    tri2_ext = nc.declare_dram_parameter("tri2", [128, 256], BF16, isOutput=False)
    ones_ext = nc.declare_dram_parameter("ones", [128, 128], BF16, isOutput=False)
    out_ext = nc.declare_dram_parameter("out", [UNITS, G, D, S], F32, isOutput=True)

    exp = mybir.ActivationFunctionType.Exp

    with TileContext(nc) as tc:
        with (
            tc.tile_pool(name="const", bufs=1) as cpool,
            tc.tile_pool(name="pt", bufs=5) as ptpool,
            tc.tile_pool(name="acc", bufs=4) as accpool,
            tc.tile_pool(name="linv", bufs=2) as lipool,
            tc.tile_pool(name="osb", bufs=2) as opool,
            tc.tile_pool(name="st", bufs=2, space="PSUM") as stpool,
            tc.tile_pool(name="ot", bufs=2, space="PSUM") as otpool,
        ):
            tri_sb = cpool.tile([128, 128], BF16, tag="tri")
            tri2_sb = cpool.tile([128, 256], BF16, tag="tri2")
            ones_sb = cpool.tile([128, 128], BF16, tag="ones")

            # Persistent K^T / V / Q^T tiles.  v is host-preswizzled to
            # [128, kc*128+d] so each load is a plain contiguous DMA.
            kT_sb = [cpool.tile([128, S], BF16, name=f"kT{u}", tag=f"kT{u}")
                     for u in range(UNITS)]
            v_sb = [cpool.tile([128, NKC * 128], BF16, name=f"v{u}", tag=f"v{u}")
                    for u in range(UNITS)]
            qT_sb = {
                (u, g): cpool.tile([128, S], BF16, name=f"qT{u}{g}", tag=f"qT{u}{g}")
                for u in range(UNITS) for g in range(G)
            }

            # Stage DMAs in block-dependency order.  The two DMAs gating
            # the very first score matmul (kT0/qT00 cols [0:512]) go out
            # in parallel on separate engine queues so their ~650ns issue
            # costs overlap; everything else streams in behind them.
            nc.sync.dma_start(out=kT_sb[0][:, 0:512], in_=kT_ext[0][:, 0:512])
            nc.scalar.dma_start(out=qT_sb[(0, 0)][:, 0:512],
                                in_=qT_ext[0, 0][:, 0:512])
            nc.gpsimd.dma_start(out=v_sb[0][:, 0:512], in_=v_ext[0][:, 0:512])
            nc.gpsimd.dma_start(out=tri_sb[:], in_=tri_ext[:])
            nc.sync.dma_start(out=kT_sb[0][:, 512:], in_=kT_ext[0][:, 512:])
            nc.scalar.dma_start(out=qT_sb[(0, 0)][:, 512:],
                                in_=qT_ext[0, 0][:, 512:])
            nc.gpsimd.dma_start(out=v_sb[0][:, 512:], in_=v_ext[0][:, 512:])
            nc.gpsimd.dma_start(out=ones_sb[:], in_=ones_ext[:])
            nc.gpsimd.dma_start(out=tri2_sb[:], in_=tri2_ext[:])
            for g in range(1, G):
                nc.sync.dma_start(out=qT_sb[(0, g)][:], in_=qT_ext[0, g])
            nc.sync.dma_start(out=kT_sb[1][:], in_=kT_ext[1])
            nc.sync.dma_start(out=v_sb[1][:], in_=v_ext[1])
            for g in range(G):
                nc.sync.dma_start(out=qT_sb[(1, g)][:], in_=qT_ext[1, g])

            # Flattened tile stream across all blocks, software-pipelined
            # 2 tiles deep: producers (ST matmuls + exp + tri masks) lead
            # consumers (PV matmuls + acc adds) by 2 stream slots.
            blocks = [(u, g, qt)
                      for u in range(UNITS)
                      for g in range(G)
                      for qt in range(NQT)]
            stream = []  # (block_idx, tile, is_last_tile_of_block)
            for bi, (u, g, qt) in enumerate(blocks):
                tl = TILES[qt]
                for j, tile in enumerate(tl):
                    stream.append((bi, tile, j == len(tl) - 1))

            bstate = {}            # block_idx -> (acc, ot)
            produced = {}          # stream idx -> pt tile
            pending_norm = []      # (ot_idx, acc, ot, u, g, qt)
            cons_block = [-1]      # block whose consumers are running
            ot_count = [0]         # ot allocations so far (ring position)
            flush_due = [None]     # loop idx at which to emit pending norm
            loop_si = [0]          # current main-loop position

            def emit_producers(si):
                bi, tile, _ = stream[si]
                u, g, qt = blocks[bi]
                width = max(pc + SQ - sq0 for (_, sq0, pc) in tile)
                st = stpool.tile([128, 1536], F32, name="st", tag="st")
                for (kc, sq0, pc) in tile:
                    nc.tensor.matmul(
                        st[:, pc:pc + SQ - sq0],
                        kT_sb[u][:, kc * 128:(kc + 1) * 128],
                        qT_sb[(u, g)][:, qt * SQ + sq0:(qt + 1) * SQ],
                        start=True,
                        stop=True,
                    )
                pt = ptpool.tile([128, 1536], BF16, name="pt", tag="pt")
                nc.scalar.activation(pt[:, :width], st[:, :width], exp,
                                     scale=SCALE)
                # causal tri-masks for diagonal chunks; adjacent 128-col
                # mask regions merge into one [128,256] op on tri2
                regions = sorted(pc for (kc, sq0, pc) in tile if kc >= 4 * qt)
                i = 0
                while i < len(regions):
                    if i + 1 < len(regions) and regions[i + 1] == regions[i] + 128:
                        nc.gpsimd.tensor_mul(
                            pt[:, regions[i]:regions[i] + 256],
                            pt[:, regions[i]:regions[i] + 256], tri2_sb[:]
                        )
                        i += 2
                    else:
                        nc.gpsimd.tensor_mul(
                            pt[:, regions[i]:regions[i] + 128],
                            pt[:, regions[i]:regions[i] + 128], tri_sb[:]
                        )
                        i += 1
                produced[si] = pt

            def flush_norm():
                # Deferred normalize of the previous block: a ones-matmul
                # both partition-reduces acc into l AND broadcasts it over
                # the 128 output partitions; its PSUM tile comes from the
                # st pool (slot ring keeps this off the critical path).
                # The final ot * 1/l runs on GpSimd — DVE is the busier
                # engine, and the lag-3 mask schedule leaves Pool slack.
                _, acc, ot, u, g, qt = pending_norm.pop(0)
                lps = stpool.tile([128, SQ], F32, name="lps", tag="st")
                nc.tensor.matmul(lps[:], ones_sb[:], acc[:],
                                 start=True, stop=True)
                linv = lipool.tile([128, SQ], F32, name="linv", tag="linv")
                nc.vector.reciprocal_approx_fast(out=linv[:], in_=lps[:])
                osb = opool.tile([128, SQ], F32, name="osb", tag="osb")
                nc.vector.tensor_mul(osb[:], ot[:], linv[:])
                nc.sync.dma_start(
                    out=out_ext[u, g][:, qt * SQ:(qt + 1) * SQ], in_=osb[:]
                )

            def enter_block(bi, si):
                # first consumer touch of a new block: the previous block's
                # normalize is deferred one more stream slot (so the PE
                # never reaches lps before the final DVE adds land), but a
                # pending ot two ring slots back must flush NOW — its PSUM
                # slot is about to be re-waited by this block's first PV.
                if bi != cons_block[0]:
                    cons_block[0] = bi
                    while pending_norm and pending_norm[0][0] <= ot_count[0] - 2:
                        flush_norm()
                    if pending_norm:
                        flush_due[0] = loop_si[0] + 1
                    ot = otpool.tile([128, SQ], F32, name="ot", tag="ot")
                    bstate[bi] = {"acc": None, "ot": ot, "idx": ot_count[0]}
                    ot_count[0] += 1
                return bstate[bi]

            def emit_chunk(bi, pt, chunk, st8, fuse_with=None):
                u, g, qt = blocks[bi]
                nkc = 4 * qt + 4
                kc, sq0, pc = chunk
                w = SQ - sq0
                pta = pt[:, pc:pc + w]
                if kc == 0:
                    acc = accpool.tile([128, SQ], BF16, name="acc", tag="acc")
                    if fuse_with is not None:
                        # acc = pt[kc0] + pt[kc1] in one DVE op (both full
                        # width); kc1's own visit skips its add
                        _, _, pc1 = fuse_with
                        nc.vector.tensor_add(acc[:], pta,
                                             pt[:, pc1:pc1 + SQ])
                    else:
                        nc.vector.tensor_copy(acc[:], pta)
                    st8["acc"] = acc
                elif kc == 1 and fuse_with is not None:
                    pass  # folded into kc0's fused add
                elif sq0 == 0:
                    # full-width add: ping-pong into a fresh buffer so the
                    # accumulate chain never reads and writes one address
                    nxt = accpool.tile([128, SQ], BF16, name="acc", tag="acc")
                    nc.vector.tensor_add(nxt[:], st8["acc"][:], pta)
                    st8["acc"] = nxt
                else:
                    acc = st8["acc"]
                    nc.vector.tensor_add(acc[:, sq0:], acc[:, sq0:], pta)
                nc.tensor.matmul(
                    st8["ot"][:, sq0:],
                    v_sb[u][:, kc * 128:(kc + 1) * 128],
                    pta,
                    start=(kc == 0),
                    stop=(kc == nkc - 1),
                )

            def emit_clean_consumers(si):
                bi, tile, last = stream[si]
                u, g, qt = blocks[bi]
                clean = sorted(c for c in tile if c[0] < 4 * qt)
                if not clean:
                    return
                st8 = enter_block(bi, si)
                pt = produced[si]
                fuse = None
                if clean[0][0] == 0 and len(clean) > 1 and clean[1][0] == 1:
                    fuse = clean[1]
                for chunk in clean:
                    emit_chunk(bi, pt, chunk, st8, fuse_with=fuse)
                if last and all(c[0] < 4 * qt for c in tile):
                    pending_norm.append((st8["idx"], st8["acc"], st8["ot"],
                                         u, g, qt))
                    del bstate[bi]

            def emit_masked_consumers(si):
                # diagonal chunks run one pipeline slot later than clean
                # ones so the GpSimd tri-masks never stall the PE
                bi, tile, last = stream[si]
                u, g, qt = blocks[bi]
                masked = sorted(c for c in tile if c[0] >= 4 * qt)
                if not masked:
                    produced.pop(si, None)
                    return
                st8 = enter_block(bi, si)
                pt = produced.pop(si)
                for chunk in masked:
                    emit_chunk(bi, pt, chunk, st8)
                if last:
                    pending_norm.append((st8["idx"], st8["acc"], st8["ot"],
                                         u, g, qt))
                    del bstate[bi]

            n = len(stream)
            for si in range(n + 3):
                loop_si[0] = si
                if flush_due[0] is not None and si >= flush_due[0]:
                    while pending_norm:
                        flush_norm()
                    flush_due[0] = None
                if si < n:
                    emit_producers(si)
                if 0 <= si - 3:
                    emit_masked_consumers(si - 3)
                if 0 <= si - 2 < n:
                    emit_clean_consumers(si - 2)
            while pending_norm:
                flush_norm()

    # Populate .instr bytes for extended-inst InstISA subclasses (the
    # custom-DVE reciprocal) — raw Bass skips the Bacc pass that does
    # this, and walrus codegen dies with "ISA wrong length" without it.
    from concourse.library_overlay import lower_extended_insts

    lower_extended_insts(nc)
    _split_multiwait_instructions(nc)
    return nc


_NC_CACHE = None


def _get_nc():
    global _NC_CACHE
    if _NC_CACHE is None:
        _NC_CACHE = build_nc()
    return _NC_CACHE


# (b, kv) unit for each of the 16 shards; core c owns pairs 2c and 2c+1.
_PAIRS = [(p // HKV, p % HKV) for p in range(B * HKV)]


def make_in_maps(q, k, v):
    qr = np.ascontiguousarray(q, dtype=np.float32).reshape(B, S, HKV, G, D)
    kr = np.ascontiguousarray(k, dtype=np.float32).reshape(B, S, HKV, D)
    vr = np.ascontiguousarray(v, dtype=np.float32).reshape(B, S, HKV, D)
    tri = np.triu(np.ones((128, 128), np.float32)).astype(NP_BF16)
    tri2 = np.concatenate([tri, tri], axis=1)
    ones = np.ones((128, 128), NP_BF16)
    in_maps = []
    for c in range(NCORES):
        qT = np.empty((UNITS, G, D, S), NP_BF16)
        kT = np.empty((UNITS, D, S), NP_BF16)
        vv = np.empty((UNITS, 128, NKC * 128), NP_BF16)
        for u in range(UNITS):
            b, kv = _PAIRS[2 * c + u]
            qT[u] = qr[b, :, kv].transpose(1, 2, 0).astype(NP_BF16)
            kT[u] = kr[b, :, kv].T.astype(NP_BF16)
            # v_sb[p, kc*128+d] = v[kc*128+p, d]
            vv[u] = (
                vr[b, :, kv].reshape(NKC, 128, D).transpose(1, 0, 2)
                .reshape(128, NKC * D).astype(NP_BF16)
            )
        in_maps.append({"qT": qT, "kT": kT, "v": vv, "tri": tri, "tri2": tri2,
                        "ones": ones})
    return in_maps


def gather_out(results):
    out = np.empty((B * S, H * D), np.float32)
    for c in range(NCORES):
        o = results[c]["out"]
        for u in range(UNITS):
            b, kv = _PAIRS[2 * c + u]
            for g in range(G):
                h = kv * G + g
                out[b * S:(b + 1) * S, h * D:(h + 1) * D] = o[u, g].T
    return out


def kernel(q, k, v, cu_seqlens_q, cu_seqlens_k, **run_kwargs):
    cu = np.asarray(cu_seqlens_q)
    assert cu.shape[0] == B + 1 and int(cu[-1]) == B * S, (
        "kernel hardcodes 4 equal sequences of 2048"
    )
    in_maps = make_in_maps(q, k, v)
    nc = _get_nc()
    res = run_bass_kernel_spmd(nc, in_maps, core_ids=list(range(NCORES)), **run_kwargs)
    out = gather_out(res.results)
    if run_kwargs:
        return out, res
    return out


# revision 45
# speedup vs baseline: 1.0048x; 1.0048x over previous
"""Causal GQA varlen-prefill attention on 8 TRN2 NeuronCores.

Problem: B=4 sequences of S=2048, 16 Q heads, 4 KV heads (GQA group 4),
head_dim 128, fp32. Sharded across 8 cores by (batch, kv-head) unit:
16 units, 2 per core - embarrassingly parallel, no collectives.

v2 redesign (from perfetto evidence on the 255us v1):
  - ACT runs ONLY exp, in 13 wide ops per (u,g) instead of 28+32copies+
    64 DMA issues: diagonal-superblock score chunks are bin-packed into
    shared [128,<=1536] PSUM tiles with zero dead columns, so every
    ACTIVATE amortizes its ~300ns fixed cost over 1280-1536 columns.
  - PE runs ONLY the score/PV matmuls (the l-reduction ones-matmul and
    the 1/l broadcast matmul are gone): l is computed by GpSimd
    partition_all_reduce (idle engine), 1/l by the custom-DVE
    reciprocal_approx_fast, and the normalize multiply reads the PSUM
    O^T accumulator directly.
  - Causal tri-masks moved DVE -> GpSimd (in-place on the pt tile).
  - PSUM: 2x[128,1536] score tiles (6 banks) + 2x[128,512] O^T
    accumulators (2 banks) = exactly 8 banks.
  - Tile stream is software-pipelined 2 tiles deep across block
    boundaries (diagonal-chunk consumers one slot later so GpSimd masks
    never gate the PE), and the deferred 1/l normalize of block n is
    emitted at block n+1's consumer entry.

Measured: ~195-199us on hardware (baseline v1: ~255us), rel err 2.9e-3
vs the f32 reference.  Engine busy at nominal clock: PE ~146us union +
~36us dependency gaps (wall-setting), DVE ~175us, ACT ~146us (pure exp
stream), GpSimd ~65us; ~17us is fixed NEFF warmup/tail.  Note: device
clock throttling makes single runs vary up to ~20% - compare traces by
per-op durations, not wall time.
"""

import sys

if "/opt/trn_rl_repo" not in sys.path:
    sys.path.insert(0, "/opt/trn_rl_repo")

import numpy as np
import ml_dtypes

import concourse.bass as bass
import concourse.mybir as mybir
from concourse.bass_utils import run_bass_kernel_spmd
from concourse.tile import TileContext, ScopedClock

B, S, H, HKV, D = 4, 2048, 16, 4, 128
G = H // HKV
NCORES = 8
UNITS = 2            # (b, kv) units per core
SQ = 512             # q-chunk (matmul moving dim)
NQT = S // SQ        # 4 q-chunks per (unit, head)
NKC = S // 128       # 16 k-chunks of 128
SCALE = 1.0 / float(np.sqrt(D))

F32 = mybir.dt.float32
BF16 = mybir.dt.bfloat16
NP_BF16 = np.dtype(ml_dtypes.bfloat16)


def _patched_drain_and_barrier(self, tick_clock, wait_clock):
    # walrus CoreV3 rejects >1 sync-wait on one InstDrain ("Too many sync
    # wait commands"); spread the kernel-tail waits over single-wait nops.
    # Also: skip the per-semaphore clearing ritual + second barrier (the
    # NEFF executes once per load; ~9us of tail EVENT_SEMAPHOREs saved).
    drain_inst = self.nc.sync.drain()
    wait_clock.add_sem_waits(
        drain_inst.ins, ScopedClock({None: tick_clock.global_clock})
    )
    si = drain_inst.ins.sync_info
    waits = list(si.on_wait or [])
    if len(waits) > 1:
        si.on_wait = []
        for w in waits:
            nop = self.nc.sync.nop(nofuse=True)
            nsi = nop.ins.sync_info
            if nsi is None:
                nop.ins.sync_info = mybir.SyncInfo(on_wait=[w], on_update=[])
            else:
                nsi.on_wait = [w]
        self.nc.sync.drain()
    self.nc.all_engine_barrier()
    assert self.sems is not None
    popped = self.nc._tile_sem_poison_stack.pop()
    assert popped is self._sem_poison


TileContext._drain_and_barrier = _patched_drain_and_barrier

_WAIT_LIMIT = 1
_nop_counter = [0]


def _split_multiwait_instructions(nc):
    # This walrus build allows only one sync-wait command per instruction
    # (CoreV3 setupSyncWait: "Too many sync wait commands").  Hoist extra
    # waits onto same-engine nops placed immediately before the instruction.
    for fn in nc.m.functions:
        for bb in fn.blocks:
            new_list = []
            changed = False
            for inst in bb.instructions:
                si = inst.sync_info
                waits = list(si.on_wait) if si is not None and si.on_wait else []
                if len(waits) > _WAIT_LIMIT:
                    keep = waits[-_WAIT_LIMIT:]
                    for w in waits[:-_WAIT_LIMIT]:
                        _nop_counter[0] += 1
                        nop = mybir.InstNoOp(
                            name=f"I-waitnop-{_nop_counter[0]}",
                            engine=inst.engine,
                            ins=[],
                            outs=[],
                            sync_info=mybir.SyncInfo(on_wait=[w], on_update=[]),
                        )
                        nc.register_instruction(nop, overwrite=True)
                        new_list.append(nop)
                    si.on_wait = keep
                    changed = True
                new_list.append(inst)
            if changed:
                bb.instructions = new_list


# Score-chunk packing: per qt, list of PSUM tiles; each tile is a list of
# (kc, sq0, pcol): chunk kc's live q-columns [sq0:512] land at packed
# columns [pcol : pcol + 512-sq0].  Every chunk segment sits inside one
# 512-f32 PSUM bank, the packing is gap-free from column 0, and diagonal
# chunks (kc >= 4qt) get a tri-mask on their leading 128 live columns.
TILES = {
    0: [[(0, 0, 0), (3, 384, 512), (1, 128, 640), (2, 256, 1024)]],
    1: [[(0, 0, 0), (1, 0, 512), (2, 0, 1024)],
        [(3, 0, 0), (4, 0, 512), (5, 128, 1024)],
        [(6, 256, 0), (7, 384, 256)]],
    2: [[(0, 0, 0), (1, 0, 512), (2, 0, 1024)],
        [(3, 0, 0), (4, 0, 512), (5, 0, 1024)],
        [(6, 0, 0), (7, 0, 512), (8, 0, 1024)],
        [(9, 128, 0), (11, 384, 384), (10, 256, 512)]],
    3: [[(0, 0, 0), (1, 0, 512), (2, 0, 1024)],
        [(3, 0, 0), (4, 0, 512), (5, 0, 1024)],
        [(6, 0, 0), (7, 0, 512), (8, 0, 1024)],
        [(9, 0, 0), (10, 0, 512), (11, 0, 1024)],
        [(12, 0, 0), (13, 128, 512), (15, 384, 896), (14, 256, 1024)]],
}


def build_nc() -> bass.Bass:
    nc = bass.Bass()
    qT_ext = nc.declare_dram_parameter("qT", [UNITS, G, D, S], BF16, isOutput=False)
    kT_ext = nc.declare_dram_parameter("kT", [UNITS, D, S], BF16, isOutput=False)
    v_ext = nc.declare_dram_parameter("v", [UNITS, 128, NKC * 128], BF16,
                                      isOutput=False)
    tri_ext = nc.declare_dram_parameter("tri", [128, 128], BF16, isOutput=False)
    tri2_ext = nc.declare_dram_parameter("tri2", [128, 256], BF16, isOutput=False)
    ones_ext = nc.declare_dram_parameter("ones", [128, 128], BF16, isOutput=False)
    out_ext = nc.declare_dram_parameter("out", [UNITS, G, D, S], F32, isOutput=True)

    exp = mybir.ActivationFunctionType.Exp

    with TileContext(nc) as tc:
        with (
            tc.tile_pool(name="const", bufs=1) as cpool,
            tc.tile_pool(name="pt", bufs=5) as ptpool,
            tc.tile_pool(name="acc", bufs=6) as accpool,
            tc.tile_pool(name="linv", bufs=3) as lipool,
            tc.tile_pool(name="osb", bufs=3) as opool,
            tc.tile_pool(name="st", bufs=2, space="PSUM") as stpool,
            tc.tile_pool(name="ot", bufs=2, space="PSUM") as otpool,
        ):
            tri_sb = cpool.tile([128, 128], BF16, tag="tri")
            tri2_sb = cpool.tile([128, 256], BF16, tag="tri2")
            ones_sb = cpool.tile([128, 128], BF16, tag="ones")

            # Persistent K^T / V / Q^T tiles.  v is host-preswizzled to
            # [128, kc*128+d] so each load is a plain contiguous DMA.
            kT_sb = [cpool.tile([128, S], BF16, name=f"kT{u}", tag=f"kT{u}")
                     for u in range(UNITS)]
            v_sb = [cpool.tile([128, NKC * 128], BF16, name=f"v{u}", tag=f"v{u}")
                    for u in range(UNITS)]
            qT_sb = {
                (u, g): cpool.tile([128, S], BF16, name=f"qT{u}{g}", tag=f"qT{u}{g}")
                for u in range(UNITS) for g in range(G)
            }

            # Stage DMAs in block-dependency order.  The two DMAs gating
            # the very first score matmul (kT0/qT00 cols [0:512]) go out
            # in parallel on separate engine queues so their ~650ns issue
            # costs overlap; everything else streams in behind them.
            nc.sync.dma_start(out=kT_sb[0][:, 0:512], in_=kT_ext[0][:, 0:512])
            nc.scalar.dma_start(out=qT_sb[(0, 0)][:, 0:512],
                                in_=qT_ext[0, 0][:, 0:512])
            nc.gpsimd.dma_start(out=v_sb[0][:, 0:512], in_=v_ext[0][:, 0:512])
            nc.gpsimd.dma_start(out=tri_sb[:], in_=tri_ext[:])
            nc.sync.dma_start(out=kT_sb[0][:, 512:], in_=kT_ext[0][:, 512:])
            nc.scalar.dma_start(out=qT_sb[(0, 0)][:, 512:],
                                in_=qT_ext[0, 0][:, 512:])
            nc.gpsimd.dma_start(out=v_sb[0][:, 512:], in_=v_ext[0][:, 512:])
            nc.gpsimd.dma_start(out=ones_sb[:], in_=ones_ext[:])
            nc.gpsimd.dma_start(out=tri2_sb[:], in_=tri2_ext[:])
            for g in range(1, G):
                nc.sync.dma_start(out=qT_sb[(0, g)][:], in_=qT_ext[0, g])
            nc.sync.dma_start(out=kT_sb[1][:], in_=kT_ext[1])
            nc.sync.dma_start(out=v_sb[1][:], in_=v_ext[1])
            for g in range(G):
                nc.sync.dma_start(out=qT_sb[(1, g)][:], in_=qT_ext[1, g])

            # Flattened tile stream across all blocks, software-pipelined
            # 2 tiles deep: producers (ST matmuls + exp + tri masks) lead
            # consumers (PV matmuls + acc adds) by 2 stream slots.
            blocks = [(u, g, qt)
                      for u in range(UNITS)
                      for g in range(G)
                      for qt in range(NQT)]
            stream = []  # (block_idx, tile, is_last_tile_of_block)
            for bi, (u, g, qt) in enumerate(blocks):
                tl = TILES[qt]
                for j, tile in enumerate(tl):
                    stream.append((bi, tile, j == len(tl) - 1))

            bstate = {}            # block_idx -> (acc, ot)
            produced = {}          # stream idx -> pt tile
            pending_norm = []      # (ot_idx, acc, ot, u, g, qt)
            cons_block = [-1]      # block whose consumers are running
            ot_count = [0]         # ot allocations so far (ring position)
            flush_due = [None]     # loop idx at which to emit pending norm
            loop_si = [0]          # current main-loop position

            def emit_producers(si):
                bi, tile, _ = stream[si]
                u, g, qt = blocks[bi]
                width = max(pc + SQ - sq0 for (_, sq0, pc) in tile)
                st = stpool.tile([128, 1536], F32, name="st", tag="st")
                for (kc, sq0, pc) in tile:
                    nc.tensor.matmul(
                        st[:, pc:pc + SQ - sq0],
                        kT_sb[u][:, kc * 128:(kc + 1) * 128],
                        qT_sb[(u, g)][:, qt * SQ + sq0:(qt + 1) * SQ],
                        start=True,
                        stop=True,
                    )
                pt = ptpool.tile([128, 1536], BF16, name="pt", tag="pt")
                nc.scalar.activation(pt[:, :width], st[:, :width], exp,
                                     scale=SCALE)
                # causal tri-masks for diagonal chunks; adjacent 128-col
                # mask regions merge into one [128,256] op on tri2
                regions = sorted(pc for (kc, sq0, pc) in tile if kc >= 4 * qt)
                i = 0
                while i < len(regions):
                    if i + 1 < len(regions) and regions[i + 1] == regions[i] + 128:
                        nc.gpsimd.tensor_mul(
                            pt[:, regions[i]:regions[i] + 256],
                            pt[:, regions[i]:regions[i] + 256], tri2_sb[:]
                        )
                        i += 2
                    else:
                        nc.gpsimd.tensor_mul(
                            pt[:, regions[i]:regions[i] + 128],
                            pt[:, regions[i]:regions[i] + 128], tri_sb[:]
                        )
                        i += 1
                produced[si] = pt

            def flush_norm():
                # Deferred normalize of the previous block: a ones-matmul
                # both partition-reduces acc into l AND broadcasts it over
                # the 128 output partitions; its PSUM tile comes from the
                # st pool (slot ring keeps this off the critical path).
                # The final ot * 1/l runs on GpSimd — DVE is the busier
                # engine, and the lag-3 mask schedule leaves Pool slack.
                _, acc, ot, u, g, qt = pending_norm.pop(0)
                lps = stpool.tile([128, SQ], F32, name="lps", tag="st")
                nc.tensor.matmul(lps[:], ones_sb[:], acc[:],
                                 start=True, stop=True)
                linv = lipool.tile([128, SQ], F32, name="linv", tag="linv")
                nc.vector.reciprocal_approx_fast(out=linv[:], in_=lps[:])
                osb = opool.tile([128, SQ], F32, name="osb", tag="osb")
                nc.vector.tensor_mul(osb[:], ot[:], linv[:])
                nc.sync.dma_start(
                    out=out_ext[u, g][:, qt * SQ:(qt + 1) * SQ], in_=osb[:]
                )

            def enter_block(bi, si):
                # first consumer touch of a new block: the previous block's
                # normalize is deferred one more stream slot (so the PE
                # never reaches lps before the final DVE adds land), but a
                # pending ot two ring slots back must flush NOW — its PSUM
                # slot is about to be re-waited by this block's first PV.
                if bi != cons_block[0]:
                    cons_block[0] = bi
                    while pending_norm and pending_norm[0][0] <= ot_count[0] - 2:
                        flush_norm()
                    if pending_norm:
                        flush_due[0] = loop_si[0] + 1
                    ot = otpool.tile([128, SQ], F32, name="ot", tag="ot")
                    bstate[bi] = {"acc": None, "ot": ot, "idx": ot_count[0]}
                    ot_count[0] += 1
                return bstate[bi]

            def emit_chunk(bi, pt, chunk, st8, fuse_with=None):
                u, g, qt = blocks[bi]
                nkc = 4 * qt + 4
                kc, sq0, pc = chunk
                w = SQ - sq0
                pta = pt[:, pc:pc + w]
                if kc == 0:
                    acc = accpool.tile([128, SQ], BF16, name="acc", tag="acc")
                    if fuse_with is not None:
                        # acc = pt[kc0] + pt[kc1] in one DVE op (both full
                        # width); kc1's own visit skips its add
                        _, _, pc1 = fuse_with
                        nc.vector.tensor_add(acc[:], pta,
                                             pt[:, pc1:pc1 + SQ])
                    else:
                        nc.vector.tensor_copy(acc[:], pta)
                    st8["acc"] = acc
                elif kc == 1 and fuse_with is not None:
                    pass  # folded into kc0's fused add
                elif sq0 == 0:
                    # full-width add: ping-pong into a fresh buffer so the
                    # accumulate chain never reads and writes one address
                    nxt = accpool.tile([128, SQ], BF16, name="acc", tag="acc")
                    nc.vector.tensor_add(nxt[:], st8["acc"][:], pta)
                    st8["acc"] = nxt
                else:
                    acc = st8["acc"]
                    nc.vector.tensor_add(acc[:, sq0:], acc[:, sq0:], pta)
                nc.tensor.matmul(
                    st8["ot"][:, sq0:],
                    v_sb[u][:, kc * 128:(kc + 1) * 128],
                    pta,
                    start=(kc == 0),
                    stop=(kc == nkc - 1),
                )

            def emit_clean_consumers(si):
                bi, tile, last = stream[si]
                u, g, qt = blocks[bi]
                clean = sorted(c for c in tile if c[0] < 4 * qt)
                if not clean:
                    return
                st8 = enter_block(bi, si)
                pt = produced[si]
                fuse = None
                if clean[0][0] == 0 and len(clean) > 1 and clean[1][0] == 1:
                    fuse = clean[1]
                for chunk in clean:
                    emit_chunk(bi, pt, chunk, st8, fuse_with=fuse)
                if last and all(c[0] < 4 * qt for c in tile):
                    pending_norm.append((st8["idx"], st8["acc"], st8["ot"],
                                         u, g, qt))
                    del bstate[bi]

            def emit_masked_consumers(si):
                # diagonal chunks run one pipeline slot later than clean
                # ones so the GpSimd tri-masks never stall the PE
                bi, tile, last = stream[si]
                u, g, qt = blocks[bi]
                masked = sorted(c for c in tile if c[0] >= 4 * qt)
                if not masked:
                    produced.pop(si, None)
                    return
                st8 = enter_block(bi, si)
                pt = produced.pop(si)
                for chunk in masked:
                    emit_chunk(bi, pt, chunk, st8)
                if last:
                    pending_norm.append((st8["idx"], st8["acc"], st8["ot"],
                                         u, g, qt))
                    del bstate[bi]

            n = len(stream)
            for si in range(n + 3):
                loop_si[0] = si
                if flush_due[0] is not None and si >= flush_due[0]:
                    while pending_norm:
                        flush_norm()
                    flush_due[0] = None
                if si < n:
                    emit_producers(si)
                if 0 <= si - 3:
                    emit_masked_consumers(si - 3)
                if 0 <= si - 2 < n:
                    emit_clean_consumers(si - 2)
            while pending_norm:
                flush_norm()

    # Populate .instr bytes for extended-inst InstISA subclasses (the
    # custom-DVE reciprocal) — raw Bass skips the Bacc pass that does
    # this, and walrus codegen dies with "ISA wrong length" without it.
    from concourse.library_overlay import lower_extended_insts

    lower_extended_insts(nc)
    _split_multiwait_instructions(nc)
    return nc


_NC_CACHE = None


def _get_nc():
    global _NC_CACHE
    if _NC_CACHE is None:
        _NC_CACHE = build_nc()
    return _NC_CACHE


# (b, kv) unit for each of the 16 shards; core c owns pairs 2c and 2c+1.
_PAIRS = [(p // HKV, p % HKV) for p in range(B * HKV)]


def make_in_maps(q, k, v):
    qr = np.ascontiguousarray(q, dtype=np.float32).reshape(B, S, HKV, G, D)
    kr = np.ascontiguousarray(k, dtype=np.float32).reshape(B, S, HKV, D)
    vr = np.ascontiguousarray(v, dtype=np.float32).reshape(B, S, HKV, D)
    tri = np.triu(np.ones((128, 128), np.float32)).astype(NP_BF16)
    tri2 = np.concatenate([tri, tri], axis=1)
    ones = np.ones((128, 128), NP_BF16)
    in_maps = []
    for c in range(NCORES):
        qT = np.empty((UNITS, G, D, S), NP_BF16)
        kT = np.empty((UNITS, D, S), NP_BF16)
        vv = np.empty((UNITS, 128, NKC * 128), NP_BF16)
        for u in range(UNITS):
            b, kv = _PAIRS[2 * c + u]
            qT[u] = qr[b, :, kv].transpose(1, 2, 0).astype(NP_BF16)
            kT[u] = kr[b, :, kv].T.astype(NP_BF16)
            # v_sb[p, kc*128+d] = v[kc*128+p, d]
            vv[u] = (
                vr[b, :, kv].reshape(NKC, 128, D).transpose(1, 0, 2)
                .reshape(128, NKC * D).astype(NP_BF16)
            )
        in_maps.append({"qT": qT, "kT": kT, "v": vv, "tri": tri, "tri2": tri2,
                        "ones": ones})
    return in_maps


def gather_out(results):
    out = np.empty((B * S, H * D), np.float32)
    for c in range(NCORES):
        o = results[c]["out"]
        for u in range(UNITS):
            b, kv = _PAIRS[2 * c + u]
            for g in range(G):
                h = kv * G + g
                out[b * S:(b + 1) * S, h * D:(h + 1) * D] = o[u, g].T
    return out


def kernel(q, k, v, cu_seqlens_q, cu_seqlens_k, **run_kwargs):
    cu = np.asarray(cu_seqlens_q)
    assert cu.shape[0] == B + 1 and int(cu[-1]) == B * S, (
        "kernel hardcodes 4 equal sequences of 2048"
    )
    in_maps = make_in_maps(q, k, v)
    nc = _get_nc()
    res = run_bass_kernel_spmd(nc, in_maps, core_ids=list(range(NCORES)), **run_kwargs)
    out = gather_out(res.results)
    if run_kwargs:
        return out, res
    return out
